# revision 1
# baseline (speedup 1.0000x reference)
"""DN4 (retrieval_knn) Trainium2 kernel over 8 NeuronCores.

Sharding: devices 0-3 handle episode 0, devices 4-7 episode 1.  Within a
group of 4 devices the 15 queries split 4/4/4/3 and the 25 support images
7/7/7/4; every device runs an identical program over 11 image slots
(4 query + 7 support, zero padded).  Training-mode BatchNorm statistics are
exact: per-device partial sums are AllReduced across all 8 devices each layer
(query batch and support batch separately).  Support descriptors are
L2-normalized on the owning device and AllGathered within each episode
group; each device then computes cosine similarities for its queries,
per-row top-3 via the VectorE max8 instruction, and the per-(query,class)
scores.

Convs run as float32r (TF32-like, ~1e-4 rel.err) matmuls on TensorE via
shift-accumulation over the 9 taps (layer 1 uses a host-built K=27 im2col).
BN stats are taken straight from PSUM with bn_stats; BN+LeakyReLU(0.2) is a
single fused ScalarE Prelu pass (layer 1 conv is recomputed instead of its
pre-BN output being stored).
"""

import sys
import numpy as np

sys.path.insert(0, "/opt/trn_rl_repo")

import concourse.bass as bass  # noqa: E402,F401
import concourse.bacc as bacc  # noqa: E402
import concourse.mybir as mybir  # noqa: E402
import concourse.tile as tile  # noqa: E402
from concourse.bass_utils import run_bass_kernel_spmd  # noqa: E402

AF = mybir.ActivationFunctionType
ALU = mybir.AluOpType
F32 = mybir.dt.float32
F32R = mybir.dt.float32r
BF16 = mybir.dt.bfloat16
AXX = mybir.AxisListType.X

B, NQ, WAY, SHOT, C, H, W = 2, 15, 5, 5, 3, 84, 84
TOPK = 3
SLOPE = 0.2
EPS_BN = 1e-5
EPS_N2 = 1e-24

N_CORES = 8
GROUP = 4
NQL, NSL = 4, 7
NSLOT = NQL + NSL  # 11

SP1, SP2, SP3 = 86, 44, 23
S1 = 84 * 84           # 7056 conv1 stream per slot (compact im2col)
QROWS = 21             # im2col quarter = 21 rows
SQ = QROWS * 84        # 1764 cols per quarter
PAD2, PAD3 = SP2 * SP2, SP3 * SP3        # 1936, 529
HW2, HW3 = 42 * 42, 21 * 21              # 1764, 441
S3 = 21 * SP3                            # 483 conv3/4 stream per slot
NF = NSLOT * HW3                         # 4851
MSTRIDE = 2208                           # class stride in gathered support
MREAL = SHOT * HW3                       # 2205

# conv chunk groups: uniform chunks per psum group, chunk c at column 512*c
G1 = [(0, 6, 2), (12, 6, 1), (18, 3, 1)]   # (row0, rows/chunk, nchunks)
G2 = [(0, 11, 2), (22, 10, 2)]
_CACHE = {}


def _round_f32r(a):
    bits = np.ascontiguousarray(a, np.float32).view(np.uint32).astype(np.uint64)
    return ((bits + 0x800) & 0xFFFFF000).astype(np.uint32).view(np.float32)


def _class_pieces():
    pieces = []
    for dv in range(GROUP):
        lo, hi = 7 * dv, min(7 * dv + 7, WAY * SHOT)
        for w in range(WAY):
            o0, o1 = max(lo, 5 * w), min(hi, 5 * w + 5)
            if o1 > o0:
                pieces.append((dv, (o0 - 7 * dv) * HW3,
                               w * MSTRIDE + (o0 - 5 * w) * HW3,
                               (o1 - o0) * HW3))
    return pieces


def build_program():
    nc = bacc.Bacc("TRN2", target_bir_lowering=False, debug=False,
                   enable_asserts=True, num_devices=N_CORES)

    im1 = nc.dram_tensor("im1", [27, NSLOT * S1], F32R, kind="ExternalInput")
    w1c = nc.dram_tensor("w1c", [27, 128], F32R, kind="ExternalInput")
    w2d = nc.dram_tensor("w2", [64, 9 * 128], F32R, kind="ExternalInput")
    w3d = nc.dram_tensor("w3", [64, 9 * 128], F32R, kind="ExternalInput")
    w4d = nc.dram_tensor("w4", [64, 9 * 128], F32R, kind="ExternalInput")
    gb = nc.dram_tensor("gb", [64, 8], F32, kind="ExternalInput")
    masks = nc.dram_tensor("masks", [64, NSLOT], F32, kind="ExternalInput")
    scores_out = nc.dram_tensor("scores", [NQL * WAY, 1], F32,
                                kind="ExternalOutput")

    CORE_IDS = list(range(N_CORES))
    GROUPS4 = [[0, 1, 2, 3], [4, 5, 6, 7]]

    NG = {(1, "q"): B * NQ * 84 * 84, (1, "s"): B * WAY * SHOT * 84 * 84,
          (2, "q"): B * NQ * HW2, (2, "s"): B * WAY * SHOT * HW2,
          (3, "q"): B * NQ * HW3, (3, "s"): B * WAY * SHOT * HW3,
          (4, "q"): B * NQ * HW3, (4, "s"): B * WAY * SHOT * HW3}
    NCHK = {1: 16, 2: 4, 3: 1, 4: 1}   # stats chunks per slot
    CNT1 = {1: 84 * 84, 2: HW2, 3: HW3, 4: HW3}

    with tile.TileContext(nc) as tc:
        with (
            tc.tile_pool(name="p0", bufs=1) as p0,
            tc.tile_pool(name="pdbl", bufs=2) as pdbl,
            tc.tile_pool(name="psm", bufs=4) as psm,
            tc.tile_pool(name="dram", bufs=1, space="DRAM") as dram,
            tc.tile_pool(name="ppb", bufs=2, space="PSUM") as ppb,
            tc.tile_pool(name="ppb3", bufs=1, space="PSUM") as ppb3,
            tc.tile_pool(name="pps", bufs=1, space="PSUM") as pps,
        ):
            # ---------- persistent tiles ----------
            w1t = p0.tile([27, 128], F32R, tag="w1")
            nc.sync.dma_start(w1t[:], w1c[:])
            wt = {}
            for li, wsrc in ((2, w2d), (3, w3d), (4, w4d)):
                t = p0.tile([64, 9 * 128], F32R, tag=f"w{li}", name=f"w{li}t")
                nc.sync.dma_start(t[:], wsrc[:])
                wt[li] = t
            gbt = p0.tile([64, 8], F32, tag="gb")
            nc.sync.dma_start(gbt[:], gb[:])
            maskt = p0.tile([64, NSLOT], F32, tag="masks")
            nc.sync.dma_start(maskt[:], masks[:])
            ones64 = p0.tile([64, 1], BF16, tag="ones64")
            nc.vector.memset(ones64[:], 1.0)
            onesk1 = p0.tile([1, 128], F32R, tag="onesk1")
            nc.vector.memset(onesk1[:].bitcast(mybir.dt.uint32), 0x3F800000)
            ones128 = p0.tile([128, 1], F32, tag="ones128")
            nc.vector.memset(ones128[:], 1.0)

            # chained big tiles (same tag => sequential address reuse)
            raw2 = p0.tile([64, NSLOT * HW2], BF16, tag="chA")   # later: ninv
            raw3 = p0.tile([64, NF], BF16, tag="chB")            # later: f2
            raw4 = p0.tile([64, NF], BF16, tag="chC")            # later: sn
            feats = p0.tile([64, NF], BF16, tag="feats")
            t3 = p0.tile([128, NQL * WAY * 4], F32, tag="t3")
            t3q = p0.tile([128, NQL * WAY], F32, tag="t3q")
            qn = p0.tile([65, NQL * 512], F32R, tag="qn")
            scs = {li: p0.tile([64, NSLOT], F32, tag=f"sc{li}", name=f"scs{li}")
                   for li in (1, 2, 3, 4)}
            bis = {li: p0.tile([64, NSLOT], F32, tag=f"bi{li}", name=f"bis{li}")
                   for li in (1, 2, 3, 4)}
            stq = {li: p0.tile([64, NQL * NCHK[li], 6], F32, tag="stq", name=f"stq{li}")
                   for li in (1, 2, 3, 4)}
            sts = {li: p0.tile([64, NSL * NCHK[li], 6], F32, tag="sts", name=f"sts{li}")
                   for li in (1, 2, 3, 4)}

            s1sum = p0.tile([64, 80], F32, tag="s1sum")
            s1sq = p0.tile([64, 80], F32, tag="s1sq")
            ar_in, ar_out = {}, {}
            for li in (1, 2, 3, 4):
                for kind in ("q", "s"):
                    ar_in[(li, kind)] = dram.tile([64, 2], F32, tag=f"ari{li}{kind}",
                                                  name=f"arin{li}{kind}")
                    ar_out[(li, kind)] = dram.tile([64, 2], F32, tag=f"aro{li}{kind}",
                                                   name=f"arout{li}{kind}")
            ag_in = dram.tile([64, NSL * HW3], F32R)
            ag_out = dram.tile([GROUP * 64, NSL * HW3], F32R)

            # ---------- helpers ----------
            def slot_kind(n):
                return "q" if n < NQL else "s"

            def stash_of(li, n):
                return (stq[li], n) if n < NQL else (sts[li], n - NQL)

            def emit_stats_reduce(li, kind):
                st = stq[li] if kind == "q" else sts[li]
                agg = psm.tile([64, 2], F32, tag="agg")
                if li == 1 and kind == "s":
                    nc.vector.bn_aggr(agg[:], st[:, 0:32, :])
                    cnt = float(CNT1[1] * 2)
                else:
                    nc.vector.bn_aggr(agg[:], st[:])
                    cnt = float(CNT1[li] * (NQL if kind == "q" else NSL))
                sums = psm.tile([64, 2], F32, tag="sums")
                nc.vector.tensor_scalar_mul(sums[:, 0:1], agg[:, 0:1], cnt)
                m2 = psm.tile([64, 1], F32, tag="m2")
                nc.vector.tensor_tensor(out=m2[:], in0=agg[:, 0:1],
                                        in1=agg[:, 0:1], op=ALU.mult)
                nc.vector.tensor_tensor(out=sums[:, 1:2], in0=agg[:, 1:2],
                                        in1=m2[:], op=ALU.add)
                nc.vector.tensor_scalar_mul(sums[:, 1:2], sums[:, 1:2], cnt)
                if li == 1 and kind == "s":
                    asum = psm.tile([64, 1], F32, tag="asum")
                    nc.vector.reduce_sum(asum[:], s1sum[:], axis=AXX)
                    nc.vector.tensor_tensor(out=sums[:, 0:1], in0=sums[:, 0:1],
                                            in1=asum[:], op=ALU.add)
                    asq = psm.tile([64, 1], F32, tag="asq")
                    nc.vector.reduce_sum(asq[:], s1sq[:], axis=AXX)
                    nc.vector.tensor_tensor(out=sums[:, 1:2], in0=sums[:, 1:2],
                                            in1=asq[:], op=ALU.add)
                nc.sync.dma_start(ar_in[(li, kind)][:], sums[:])
                nc.gpsimd.collective_compute(
                    "AllReduce", ALU.add, replica_groups=[CORE_IDS],
                    ins=[ar_in[(li, kind)].opt()],
                    outs=[ar_out[(li, kind)].opt()])

            def emit_bn_params(li, kind):
                g = psm.tile([64, 2], F32, tag="gsum")
                nc.sync.dma_start(g[:], ar_out[(li, kind)][:])
                inv_n = 1.0 / NG[(li, kind)]
                mean = psm.tile([64, 1], F32, tag="mean")
                ex2 = psm.tile([64, 1], F32, tag="ex2")
                nc.vector.tensor_scalar_mul(mean[:], g[:, 0:1], inv_n)
                nc.vector.tensor_scalar_mul(ex2[:], g[:, 1:2], inv_n)
                var = psm.tile([64, 1], F32, tag="var")
                nc.vector.tensor_tensor(out=var[:], in0=mean[:], in1=mean[:],
                                        op=ALU.mult)
                nc.vector.tensor_tensor(out=var[:], in0=ex2[:], in1=var[:],
                                        op=ALU.subtract)
                nc.vector.tensor_scalar_add(var[:], var[:], EPS_BN)
                lnv = psm.tile([64, 1], F32, tag="lnv")
                nc.scalar.activation(lnv[:], var[:], AF.Ln)
                rstd = psm.tile([64, 1], F32, tag="rstd")
                nc.scalar.activation(rstd[:], lnv[:], AF.Exp, scale=-0.5)
                sc_ch = psm.tile([64, 1], F32, tag="scch")
                nc.vector.tensor_tensor(out=sc_ch[:], in0=rstd[:],
                                        in1=gbt[:, 2 * li - 2:2 * li - 1],
                                        op=ALU.mult)
                bi_ch = psm.tile([64, 1], F32, tag="bich")
                nc.vector.tensor_tensor(out=bi_ch[:], in0=mean[:],
                                        in1=sc_ch[:], op=ALU.mult)
                nc.vector.tensor_tensor(out=bi_ch[:],
                                        in0=gbt[:, 2 * li - 1:2 * li],
                                        in1=bi_ch[:], op=ALU.subtract)
                cols = slice(0, NQL) if kind == "q" else slice(NQL, NSLOT)
                nc.vector.tensor_scalar(out=scs[li][:, cols],
                                        in0=maskt[:, cols], scalar1=sc_ch[:],
                                        scalar2=None, op0=ALU.mult)
                nc.vector.tensor_scalar(out=bis[li][:, cols],
                                        in0=maskt[:, cols], scalar1=bi_ch[:],
                                        scalar2=None, op0=ALU.mult)

            def gview(ps, nch, nr, sp, vw):
                """(64, nch, nr, vw) valid view of a psum group tile whose
                chunk c sits at column 512*c."""
                z = ps[0:64, 0:512 * nch].rearrange("p (c z) -> p c z", z=512)
                return z[:, :, 0:nr * sp].rearrange(
                    "p c (r x) -> p c r x", x=sp)[:, :, :, 0:vw]

            def im_quarter(n, qt):
                t = pdbl.tile([27, SQ], F32R, tag="im")
                c0 = n * S1 + qt * SQ
                nc.sync.dma_start(t[:], im1[:, c0:c0 + SQ])
                return t

            # =========================================================
            # layer 1 pass 1: conv + stats only
            # =========================================================
            ACT_STATS = set(range(6, NSLOT))      # 5 support slots -> ACT
            for n in range(NSLOT):
                st, sl = stash_of(1, n)
                for qt in range(4):
                    ib = im_quarter(n, qt)
                    for gi, (gr0, nr, nch) in enumerate(G1):
                        ps = ppb.tile([128, 1024], F32, tag="pb")
                        for c in range(nch):
                            r0 = gr0 + c * nr
                            nc.tensor.matmul(
                                ps[:, 512 * c:512 * c + nr * 84], w1t[:],
                                ib[:, r0 * 84:(r0 + nr) * 84],
                                start=True, stop=True)
                        g0 = sl * 16 + qt * 4 + [0, 2, 3][gi]
                        for c in range(nch):
                            pv = ps[0:64, 512 * c:512 * c + nr * 84]
                            if n in ACT_STATS:
                                k = (n - 6) * 16 + qt * 4 + [0, 2, 3][gi] + c
                                scr = psm.tile([64, 512], F32, tag="scr")
                                nc.scalar.activation(
                                    scr[:, 0:nr * 84], pv, AF.Copy,
                                    accum_out=s1sum[:, k:k + 1])
                                scr2 = psm.tile([64, 512], F32, tag="scr2")
                                nc.scalar.activation(
                                    scr2[:, 0:nr * 84], pv, AF.Square,
                                    accum_out=s1sq[:, k:k + 1])
                            else:
                                nc.vector.bn_stats(
                                    st[:, g0 + c:g0 + c + 1, :], pv)
                if n == NQL - 1:
                    emit_stats_reduce(1, "q")
                if n == NSLOT - 1:
                    emit_stats_reduce(1, "s")
            emit_bn_params(1, "q")
            emit_bn_params(1, "s")

            # =========================================================
            # layer 1 pass 2 (+BN+pool) then layer 2 conv, per slot
            # =========================================================
            for n in range(NSLOT):
                l2s = pdbl.tile([64, PAD2 + 8], F32R, tag="l2in")
                nc.gpsimd.memset(l2s[:].bitcast(mybir.dt.uint32), 0)
                for hh in range(2):              # half slot = 42 rows
                    bn1 = pdbl.tile([64, 42 * 84], BF16, tag="bn1")
                    for qq in range(2):
                        qt = 2 * hh + qq
                        ib = im_quarter(n, qt)
                        for (gr0, nr, nch) in G1:
                            ps = ppb.tile([128, 1024], F32, tag="pb")
                            for c in range(nch):
                                r0 = gr0 + c * nr
                                nc.tensor.matmul(
                                    ps[:, 512 * c:512 * c + nr * 84],
                                    w1t[:], ib[:, r0 * 84:(r0 + nr) * 84],
                                    start=True, stop=True)
                            v = ps[0:64, 0:nch * 512].rearrange(
                                "p (c z) -> p c z", z=512)[:, :, 0:nr * 84]
                            b0 = (qq * QROWS + gr0) * 84
                            o = bn1[:, b0:b0 + nch * nr * 84].rearrange(
                                "p (c z) -> p c z", z=nr * 84)
                            nc.scalar.activation(
                                o, v, AF.Prelu, bias=bis[1][:, n:n + 1],
                                scale=scs[1][:, n:n + 1], alpha=SLOPE)
                    hp = pdbl.tile([64, 42 * 42], BF16, tag="hp1")
                    bv = bn1[:].rearrange("p (r x two) -> p r x two",
                                          x=42, two=2)
                    nc.vector.tensor_tensor(
                        out=hp[:].rearrange("p (r x) -> p r x", x=42),
                        in0=bv[:, :, :, 0], in1=bv[:, :, :, 1], op=ALU.max)
                    hv = hp[:].rearrange("p (r two x) -> p r two x",
                                         two=2, x=42)
                    dst = l2s[:, 0:PAD2].rearrange(
                        "p (h w) -> p h w", w=SP2)[:, 1 + 21 * hh:22 + 21 * hh,
                                                   1:43]
                    nc.vector.tensor_tensor(out=dst, in0=hv[:, :, 0, :],
                                            in1=hv[:, :, 1, :], op=ALU.max)
                # ---- layer 2 conv on this slot ----
                st, sl = stash_of(2, n)
                for (gr0, nr, nch) in G2:
                    ps = ppb.tile([128, 1024], F32, tag="pb")
                    for c in range(nch):
                        r0 = gr0 + c * nr
                        for t in range(9):
                            off = r0 * SP2 + (t // 3) * SP2 + (t % 3)
                            nc.tensor.matmul(
                                ps[:, 512 * c:512 * c + nr * SP2],
                                wt[2][:, 128 * t:128 * t + 128],
                                l2s[:, off:off + nr * SP2],
                                start=(t == 0), stop=(t == 8))
                    v = gview(ps, nch, nr, SP2, 42)
                    o = raw2[:, n * HW2 + gr0 * 42:
                             n * HW2 + (gr0 + nch * nr) * 42].rearrange(
                        "p (c r x) -> p c r x", r=nr, x=42)
                    nc.scalar.copy(o, v)
                for c4 in range(4):
                    nc.vector.bn_stats(
                        st[:, sl * 4 + c4:sl * 4 + c4 + 1, :],
                        raw2[:, n * HW2 + c4 * 441:n * HW2 + (c4 + 1) * 441])
                if n == NQL - 1:
                    emit_stats_reduce(2, "q")
                if n == NSLOT - 1:
                    emit_stats_reduce(2, "s")
            emit_bn_params(2, "q")
            emit_bn_params(2, "s")

            # =========================================================
            # layer 2 BN + pool -> layer 3 conv, per slot
            # =========================================================
            def conv33(li, src_tile, n, raw_dst):
                st, sl = stash_of(li, n)
                ps = ppb.tile([128, 1024], F32, tag="pb")
                for t in range(9):
                    off = (t // 3) * SP3 + (t % 3)
                    nc.tensor.matmul(ps[:, 0:484],
                                     wt[li][:, 128 * t:128 * t + 128],
                                     src_tile[:, off:off + 484],
                                     start=(t == 0), stop=(t == 8))
                v = ps[0:64, 0:S3].rearrange("p (r x) -> p r x",
                                             x=SP3)[:, :, 0:21]
                o = raw_dst[:, n * HW3:(n + 1) * HW3].rearrange(
                    "p (r x) -> p r x", x=21)
                nc.scalar.copy(o, v)
                nc.vector.bn_stats(st[:, sl:sl + 1, :],
                                   raw_dst[:, n * HW3:(n + 1) * HW3])

            for n in range(NSLOT):
                bn2 = pdbl.tile([64, HW2], BF16, tag="bn1")
                nc.scalar.activation(
                    bn2[:], raw2[:, n * HW2:(n + 1) * HW2], AF.Prelu,
                    bias=bis[2][:, n:n + 1], scale=scs[2][:, n:n + 1],
                    alpha=SLOPE)
                l3s = pdbl.tile([64, PAD3 + 8], F32R, tag="l3in")
                nc.gpsimd.memset(l3s[:].bitcast(mybir.dt.uint32), 0)
                hp = pdbl.tile([64, 42 * 21], BF16, tag="hp1")
                bv = bn2[:].rearrange("p (r x two) -> p r x two", x=21, two=2)
                nc.vector.tensor_tensor(
                    out=hp[:].rearrange("p (r x) -> p r x", x=21),
                    in0=bv[:, :, :, 0], in1=bv[:, :, :, 1], op=ALU.max)
                hv = hp[:].rearrange("p (r two x) -> p r two x", two=2, x=21)
                dst = l3s[:, 0:PAD3].rearrange(
                    "p (h w) -> p h w", w=SP3)[:, 1:22, 1:22]
                nc.vector.tensor_tensor(out=dst, in0=hv[:, :, 0, :],
                                        in1=hv[:, :, 1, :], op=ALU.max)
                conv33(3, l3s, n, raw3)
                if n == NQL - 1:
                    emit_stats_reduce(3, "q")
                if n == NSLOT - 1:
                    emit_stats_reduce(3, "s")
            emit_bn_params(3, "q")
            emit_bn_params(3, "s")

            # ---------- layer 3 BN -> layer 4 conv ----------
            for n in range(NSLOT):
                l4s = pdbl.tile([64, PAD3 + 8], F32R, tag="l3in")
                nc.gpsimd.memset(l4s[:].bitcast(mybir.dt.uint32), 0)
                dst = l4s[:, 0:PAD3].rearrange(
                    "p (h w) -> p h w", w=SP3)[:, 1:22, 1:22]
                src = raw3[:, n * HW3:(n + 1) * HW3].rearrange(
                    "p (h w) -> p h w", w=21)
                nc.scalar.activation(dst, src, AF.Prelu,
                                     bias=bis[3][:, n:n + 1],
                                     scale=scs[3][:, n:n + 1], alpha=SLOPE)
                conv33(4, l4s, n, raw4)
                if n == NQL - 1:
                    emit_stats_reduce(4, "q")
                if n == NSLOT - 1:
                    emit_stats_reduce(4, "s")
            emit_bn_params(4, "q")
            emit_bn_params(4, "s")

            for n in range(NSLOT):
                nc.scalar.activation(
                    feats[:, n * HW3:(n + 1) * HW3],
                    raw4[:, n * HW3:(n + 1) * HW3], AF.Prelu,
                    bias=bis[4][:, n:n + 1], scale=scs[4][:, n:n + 1],
                    alpha=SLOPE)

            # =========================================================
            # L2-normalize descriptors, AllGather support
            # =========================================================
            f2 = p0.tile([64, NF], BF16, tag="chB")
            nc.scalar.activation(f2[:], feats[:], AF.Square)
            n2c = p0.tile([1, NF + 1], F32R, tag="chC")
            nc.vector.memset(n2c[:, NF:NF + 1].bitcast(mybir.dt.uint32), 0x3F800000)
            for c0 in range(0, NF, 512):
                cn = min(512, NF - c0)
                ps = pps.tile([128, 512], F32, tag="ps")
                nc.tensor.matmul(ps[0:1, 0:cn], ones64[:], f2[:, c0:c0 + cn],
                                 start=True, stop=True)
                nc.vector.tensor_scalar(out=n2c[:, c0:c0 + cn],
                                        in0=ps[0:1, 0:cn], scalar1=EPS_N2,
                                        scalar2=None, op0=ALU.max)
            ninv = p0.tile([64, NF], F32, tag="chA")
            for c0 in range(0, NF, 512):
                cn = min(512, NF - c0)
                cne = cn + (cn % 2)
                ps = pps.tile([128, 512], F32, tag="ps")
                nc.tensor.matmul(ps[:, 0:cne], onesk1[:], n2c[:, c0:c0 + cne],
                                 start=True, stop=True)
                lnb = psm.tile([64, 512], F32, tag="lnb")
                nc.scalar.activation(lnb[:, 0:cn], ps[0:64, 0:cn], AF.Ln)
                nc.scalar.activation(ninv[:, c0:c0 + cn], lnb[:, 0:cn],
                                     AF.Exp, scale=-0.5)
            nc.gpsimd.memset(qn[:].bitcast(mybir.dt.uint32), 0)
            nc.vector.memset(qn[64:65, :].bitcast(mybir.dt.uint32), 0x3F800000)
            qsrc = feats[:, 0:NQL * HW3].rearrange("p (n l) -> p n l", l=HW3)
            qninv = ninv[:, 0:NQL * HW3].rearrange("p (n l) -> p n l", l=HW3)
            qdst = qn[0:64, :].rearrange("p (n l) -> p n l", l=512)[:, :, 0:HW3]
            nc.vector.tensor_tensor(out=qdst, in0=qsrc, in1=qninv, op=ALU.mult)
            sn = p0.tile([64, NSL * HW3], F32R, tag="chC")
            nc.vector.tensor_tensor(out=sn[:], in0=feats[:, NQL * HW3:],
                                    in1=ninv[:, NQL * HW3:], op=ALU.mult)
            nc.sync.dma_start(ag_in[:], sn[:])
            nc.gpsimd.collective_compute(
                "AllGather", ALU.bypass, replica_groups=GROUPS4,
                ins=[ag_in.opt()], outs=[ag_out.opt()])

            sg = p0.tile([65, WAY * MSTRIDE], F32R, tag="chA")
            nc.gpsimd.memset(sg[:].bitcast(mybir.dt.uint32), 0)
            for w in range(WAY):
                nc.vector.memset(
                    sg[64:65, w * MSTRIDE + MREAL:(w + 1) * MSTRIDE].bitcast(
                        mybir.dt.uint32), 0xC61C4000)
            for (dv, sc0, dc0, ncols) in _class_pieces():
                nc.sync.dma_start(sg[0:64, dc0:dc0 + ncols],
                                  ag_out[64 * dv:64 * dv + 64, sc0:sc0 + ncols])

            # =========================================================
            # similarity + top-3 + scores
            # =========================================================
            for qi in range(NQL):
                for w in range(WAY):
                    for j in range(4):
                        lhs = qn[:, 512 * qi + 128 * j:512 * qi + 128 * (j + 1)]
                        sim_sb = psm.tile([128, MREAL], BF16, tag="simsb",
                                          bufs=2)
                        psa = ppb.tile([128, 1024], F32, tag="pb")
                        for ncb in range(2):
                            c0 = w * MSTRIDE + 512 * ncb
                            nc.tensor.matmul(
                                psa[:, 512 * ncb:512 * (ncb + 1)], lhs,
                                sg[:, c0:c0 + 512], start=True, stop=True)
                        nc.scalar.copy(sim_sb[:, 0:1024], psa[:])
                        psb = ppb3.tile([128, 1536], F32, tag="pb3")
                        for ncb in range(2):
                            c0 = w * MSTRIDE + 1024 + 512 * ncb
                            nc.tensor.matmul(
                                psb[:, 512 * ncb:512 * (ncb + 1)], lhs,
                                sg[:, c0:c0 + 512], start=True, stop=True)
                        c0 = w * MSTRIDE + 2048
                        nc.tensor.matmul(psb[:, 1024:1182], lhs,
                                         sg[:, c0:c0 + 158],
                                         start=True, stop=True)
                        nc.scalar.copy(sim_sb[:, 1024:MREAL], psb[:, 0:1181])
                        top8 = psm.tile([128, 8], BF16, tag="top8")
                        nc.vector.max(top8[:], sim_sb[:])
                        col = (qi * WAY + w) * 4 + j
                        nc.vector.reduce_sum(t3[:, col:col + 1],
                                             top8[:, 0:TOPK], axis=AXX)
            nc.vector.reduce_sum(
                t3q[:], t3[:].rearrange("p (c j) -> p c j", j=4), axis=AXX)
            ps = pps.tile([128, 512], F32, tag="ps")
            nc.tensor.matmul(ps[0:NQL * WAY, 0:1], t3q[:], ones128[:],
                             start=True, stop=True)
            sc_sb = psm.tile([NQL * WAY, 1], F32, tag="scout")
            nc.scalar.copy(sc_sb[:], ps[0:NQL * WAY, 0:1])
            nc.sync.dma_start(scores_out[:], sc_sb[:])

    nc.compile()
    return nc


def _prep_inputs(query, support, W1, W2, W3, W4, g1, b1, g2, b2, g3, b3,
                 g4, b4):
    query = np.asarray(query, np.float32)
    support = np.asarray(support, np.float32)
    Ws = [np.asarray(w, np.float32) for w in (W1, W2, W3, W4)]
    gs = [np.asarray(g, np.float32) for g in (g1, g2, g3, g4)]
    bs = [np.asarray(b, np.float32) for b in (b1, b2, b3, b4)]

    w1b = Ws[0].transpose(1, 2, 3, 0).reshape(27, 64)
    w1c = _round_f32r(np.concatenate([w1b, w1b], axis=1))
    wl = {}
    for li, Wm in ((2, Ws[1]), (3, Ws[2]), (4, Ws[3])):
        m = Wm.transpose(2, 3, 1, 0).reshape(9, 64, 64)
        m = np.concatenate([m, m], axis=2)              # (9, 64, 128)
        wl[li] = _round_f32r(np.ascontiguousarray(
            m.transpose(1, 0, 2)).reshape(64, 9 * 128))
    gbm = np.stack([gs[0], bs[0], gs[1], bs[1], gs[2], bs[2], gs[3], bs[3]],
                   axis=1).astype(np.float32)

    sflat = support.reshape(B, WAY * SHOT, C, H, W)
    in_maps, meta = [], []
    for d in range(N_CORES):
        e, g = d // GROUP, d % GROUP
        q0, q1 = 4 * g, min(4 * g + 4, NQ)
        s0, s1 = 7 * g, min(7 * g + 7, WAY * SHOT)
        slots = np.zeros((NSLOT, C, H, W), np.float32)
        slots[0:q1 - q0] = query[e, q0:q1]
        slots[NQL:NQL + s1 - s0] = sflat[e, s0:s1]
        mask = np.zeros(NSLOT, np.float32)
        mask[0:q1 - q0] = 1.0
        mask[NQL:NQL + s1 - s0] = 1.0

        padded = np.zeros((NSLOT, C, H + 2, W + 2), np.float32)
        padded[:, :, 1:85, 1:85] = slots
        cols = np.empty((C, 3, 3, NSLOT, 84, 84), np.float32)
        for dy in range(3):
            for dx in range(3):
                cols[:, dy, dx] = padded[:, :, dy:dy + 84,
                                         dx:dx + 84].transpose(1, 0, 2, 3)
        im2col = _round_f32r(np.ascontiguousarray(cols).reshape(
            27, NSLOT * S1))

        in_maps.append({
            "im1": im2col, "w1c": w1c, "w2": wl[2], "w3": wl[3], "w4": wl[4],
            "gb": gbm, "masks": np.broadcast_to(mask, (64, NSLOT)).copy(),
        })
        meta.append((e, q0, q1))
    return in_maps, meta


def kernel(**inputs) -> np.ndarray:
    if "nc" not in _CACHE:
        _CACHE["nc"] = build_program()
    nc = _CACHE["nc"]
    in_maps, meta = _prep_inputs(**inputs)
    res = run_bass_kernel_spmd(nc, in_maps, list(range(N_CORES)))
    out = np.zeros((B * NQ, WAY), np.float32)
    for d in range(N_CORES):
        e, q0, q1 = meta[d]
        sc = res.results[d]["scores"].reshape(NQL, WAY)
        out[e * NQ + q0:e * NQ + q1] = sc[0:q1 - q0]
    return out

